# revision 13
# baseline (speedup 1.0000x reference)
"""Pairwise squared-distance kernel for Trainium2 (8 NeuronCores).

out[i, j] = mean_d (x_i[d] - y_j[d])^2
          = (||x_i||^2 + ||y_j||^2 - 2 x_i . y_j) / D

Sharding: rows of z_queries split across 8 cores (1024 rows each);
class_prototypes replicated. Each core computes its [1024, 4096] slab.

fp8 device kernel (per core), COMPUTE_DT="fp8":
  - inputs quantized to fp8 e4m3 with asymmetric scales (x * -2^-4,
    y * 2^-4) so PSUM accumulates -2*x.y/D directly without pushing
    small values into the fp8 subnormal range.
  - GEMM in DoubleRow perf mode: lhsT [128,(2,128)] / rhs [128,(2,512)]
    contract K=256 per matmul; 2 matmuls per [128,512] psum half.
  - PE warm-up: a short stream of zero dummy matmuls starts the DVFS
    ramp while the first input DMA is in flight.
  - c0 split so the 160 KiB needed by the first real matmul lands first.
  - epilogue split 3-way (Act / DVE / Pool), one op per [128,1024] psum
    tile (Activation Identity: psum*S + (a_i-1)*S, or tensor_scalar
    mult/add), output int8 (S=120); norms a_i exact from host fp64.
  - output DMA'd per psum tile ([128,1024] int8) on the DVE/Pool queues
    (input owns the sync queue); host dequantizes q/S + 1 + b_j.
"""

import sys

if "/opt/trn_rl_repo" not in sys.path:
    sys.path.insert(0, "/opt/trn_rl_repo")

import numpy as np

N_CORES = 8
N_Q = 8192
N_P = 4096
D = 512
ROWS = N_Q // N_CORES  # 1024 query rows per core
P = 128
M_TILES = ROWS // P  # 8
NB = 512  # matmul moving free dim per k-slot (1 psum bank fp32)
NBLK = N_P // NB  # 8 column blocks of 512
QSCALE = 2.0**-4  # asymmetric fp8 pre-scale; product carries -2/D = -2^-8
OSCALE = 120.0  # int8 output scale: q = (psum + a_i - 1) * OSCALE

COMPUTE_DT = "fp8"

N_DUMMY = 10  # PE warm-up matmuls while first input DMA is in flight

_CACHE = {}


def _build_nc_fp8():
    import concourse.mybir as mybir
    import concourse.tile as tile
    from concourse import bacc

    fp8 = mybir.dt.float8e4
    f32 = mybir.dt.float32
    i8 = mybir.dt.int8
    DR = mybir.MatmulPerfMode.DoubleRow

    nc = bacc.Bacc("TRN2", target_bir_lowering=False, debug=False, num_devices=N_CORES)

    # DoubleRow k-packing: k = j*256 + s*128 + p  (j: double-tile, s: slot)
    # Input pieces in exact consumption order of the first (p=0) column-block
    # pass so the slow-clock-era matmul stream is never input-starved:
    #   xa = [q j0 h0 m0 | p b0 j0]     xb = [p b1 j0 | q j1 h0 m0]
    #   xc = [p b0 j1]                  xd = [p b1 j1]
    #   xe = [q j0 h0 m1-3 | q j1 h0 m1-3]
    #   xf = [q j0 h1 | q j1 h1]
    xa = nc.dram_tensor("xa", (P, 2, 128 + NB), fp8, kind="ExternalInput")
    xb = nc.dram_tensor("xb", (P, 2, NB + 128), fp8, kind="ExternalInput")
    xc = nc.dram_tensor("xc", (P, 2, NB), fp8, kind="ExternalInput")
    xd = nc.dram_tensor("xd", (P, 2, NB), fp8, kind="ExternalInput")
    xe = nc.dram_tensor("xe", (P, 2, 2 * 384), fp8, kind="ExternalInput")
    xf = nc.dram_tensor("xf", (P, 2, 2 * NB), fp8, kind="ExternalInput")
    pbulk = nc.dram_tensor("pbulk", (P, 6, 2, 2, NB), fp8, kind="ExternalInput")
    ab = nc.dram_tensor("ab", (P, M_TILES), f32, kind="ExternalInput")  # (a-1)*S
    out = nc.dram_tensor("out", (ROWS, N_P), i8, kind="ExternalOutput")

    with tile.TileContext(nc) as tc:
        with (
            tc.tile_pool(name="inputs", bufs=1) as in_pool,
            tc.tile_pool(name="outs", bufs=8) as out_pool,
            tc.tile_pool(name="psum", bufs=4, space="PSUM") as psum_pool,
        ):
            pt = [None, None]  # bulk tiles: [b2,b3], [b4..b7]

            # dummy operands for the PE warm-up stream (zeros)
            dw = in_pool.tile([P, 2, 128], fp8, name="dw")
            nc.vector.memset(dw, 0.0)
            dm = in_pool.tile([P, 2, 128], fp8, name="dm")
            nc.vector.memset(dm, 0.0)

            # preload the Scalar engine's activation table while idle so the
            # first real Identity epilogue doesn't eat the 1.3us table load
            warm_t = in_pool.tile([P, 1], f32, name="warm")
            nc.vector.memset(warm_t, 0.0)
            warm_o = in_pool.tile([P, 1], f32, name="warm_o")
            nc.scalar.activation(
                warm_o, warm_t, func=mybir.ActivationFunctionType.Identity
            )

            # input stream: strictly ordered on the sync queue (q1); ab on q10
            ta = in_pool.tile([P, 2, 128 + NB], fp8, name="ta")
            nc.sync.dma_start(out=ta, in_=xa[:, :])
            tb = in_pool.tile([P, 2, NB + 128], fp8, name="tb")
            nc.sync.dma_start(out=tb, in_=xb[:, :])
            ab_t = in_pool.tile([P, M_TILES], f32, name="ab_t")
            nc.scalar.dma_start(out=ab_t, in_=ab[:, :])
            tc_ = in_pool.tile([P, 2, NB], fp8, name="tc_")
            nc.sync.dma_start(out=tc_, in_=xc[:, :])
            td = in_pool.tile([P, 2, NB], fp8, name="td")
            nc.sync.dma_start(out=td, in_=xd[:, :])
            te = in_pool.tile([P, 2, 2 * 384], fp8, name="te")
            nc.sync.dma_start(out=te, in_=xe[:, :])
            tf = in_pool.tile([P, 2, 2 * NB], fp8, name="tf")
            nc.sync.dma_start(out=tf, in_=xf[:, :])
            pt[0] = in_pool.tile([P, 2, 2, 2, NB], fp8, name="pb0")
            nc.sync.dma_start(out=pt[0], in_=pbulk[:, 0:2])
            pt[1] = in_pool.tile([P, 4, 2, 2, NB], fp8, name="pb1")
            nc.sync.dma_start(out=pt[1], in_=pbulk[:, 2:6])

            # PE warm-up: a gapless stream of zero matmuls bridges the Tensor
            # engine from the preamble barrier to the first real matmul so
            # the DVFS ramp starts as early as possible
            dummy_ps = psum_pool.tile([P, 2 * NB], f32, name="ps", tag="ps")
            for _ in range(N_DUMMY):
                nc.tensor.matmul(
                    dummy_ps[:, 0:128], dw, dm, start=True, stop=True, perf_mode=DR
                )

            def rhs_ap(b, j):
                if b == 0:
                    return ta[:, :, 128 : 128 + NB] if j == 0 else tc_
                if b == 1:
                    return tb[:, :, 0:NB] if j == 0 else td
                if b < 4:
                    return pt[0][:, b - 2, j]
                return pt[1][:, b - 4, j]

            def lhs_ap(j, m):
                h, mm_ = divmod(m, 4)
                if h == 0:
                    if mm_ == 0:
                        return (
                            ta[:, :, 0:128] if j == 0 else tb[:, :, NB : NB + 128]
                        )
                    off = 0 if j == 0 else 384
                    return te[:, :, off + (mm_ - 1) * 128 : off + mm_ * 128]
                off = 0 if j == 0 else NB
                return tf[:, :, off + mm_ * P : off + (mm_ + 1) * P]

            def epi_act(dst, ps, m):
                nc.scalar.activation(
                    dst,
                    ps,
                    func=mybir.ActivationFunctionType.Identity,
                    bias=ab_t[:, m : m + 1],
                    scale=float(OSCALE),
                )

            def _epi_ts(eng, dst, ps, m):
                eng.tensor_scalar(
                    out=dst,
                    in0=ps,
                    scalar1=float(OSCALE),
                    scalar2=ab_t[:, m : m + 1],
                    op0=mybir.AluOpType.mult,
                    op1=mybir.AluOpType.add,
                )

            def epi_dve(dst, ps, m):
                _epi_ts(nc.vector, dst, ps, m)

            # epilogue split Act/DVE greedily (GpSimd cannot access PSUM);
            # output-DMA triggers: first 8 tiles on scalar (q10) while the
            # input stream still owns the sync queue (q1), then alternate
            # sync (q1) / gpsimd (q0, SW-DGE at ~670ns) to spread transfers
            epi_busy = {"A": 0.0, "D": 0.0}
            epi_cost = {"A": 1150.0, "D": 1290.0}
            epi_fn = {"A": epi_act, "D": epi_dve}
            trig_eng = {"A": nc.scalar, "S": nc.sync, "P": nc.gpsimd}
            N_TILES = 4 * M_TILES
            epi_seq = []
            trig_seq = []
            for idx in range(N_TILES - 1):
                e = min(epi_busy, key=lambda k: epi_busy[k] + epi_cost[k])
                epi_busy[e] += epi_cost[e]
                epi_seq.append(e)
                if idx < 8:
                    trig_seq.append("A")
                    epi_busy["A"] += 607.0
                else:
                    trig_seq.append("S" if idx % 2 == 0 else "P")

            gi = 0
            for p in range(4):  # column-block pair (2p, 2p+1)
                for m in range(M_TILES):
                    ps = psum_pool.tile([P, 2 * NB], f32, name="ps", tag="ps")
                    for j in (0, 1):
                        lw = lhs_ap(j, m)
                        for i in (0, 1):
                            nc.tensor.matmul(
                                ps[:, i * NB : (i + 1) * NB],
                                lw,
                                rhs_ap(2 * p + i, j),
                                start=(j == 0),
                                stop=(j == 1),
                                perf_mode=DR,
                            )
                    row = slice(m * P, (m + 1) * P)
                    col = slice(p * 2 * NB, (p + 1) * 2 * NB)
                    last = p == 3 and m == M_TILES - 1
                    if last:
                        # split the final epilogue across both psum engines
                        # into two INDEPENDENT out tiles (a shared tile
                        # falsely serializes the two writers), each DMA'd on
                        # its own queue as soon as its half is done
                        oa = out_pool.tile([P, NB], i8, name="oa")
                        od = out_pool.tile([P, NB], i8, name="od")
                        epi_dve(od, ps[:, NB:], m)
                        epi_act(oa, ps[:, 0:NB], m)
                        nc.sync.dma_start(
                            out=out[row, p * 2 * NB + NB : (p + 1) * 2 * NB],
                            in_=od,
                        )
                        nc.scalar.dma_start(
                            out=out[row, p * 2 * NB : p * 2 * NB + NB],
                            in_=oa,
                        )
                    else:
                        o = out_pool.tile([P, 2 * NB], i8, name="ot")
                        epi_fn[epi_seq[gi]](o, ps, m)
                        trig_eng[trig_seq[gi]].dma_start(out=out[row, col], in_=o)
                        gi += 1

    nc.compile()
    return nc


def _prep_inputs_fp8(z_queries, class_prototypes):
    import ml_dtypes

    e4 = ml_dtypes.float8_e4m3

    z = np.ascontiguousarray(z_queries, dtype=np.float32)
    pr = np.ascontiguousarray(class_prototypes, dtype=np.float32)

    a = (z.astype(np.float64) ** 2).sum(axis=1) / D  # (N_Q,) ||x||^2 / D
    b = (pr.astype(np.float64) ** 2).sum(axis=1) / D  # (N_P,) ||y||^2 / D

    ys8 = (pr * np.float32(QSCALE)).astype(e4)  # [N_P, D]
    # yq[j, s, p, b, c] = ys8[b*512+c, j*256+s*128+p]
    yq = ys8.T.reshape(2, 2, P, NBLK, NB)
    # pbulk[p, b-2, j, s, c]
    pbulk = np.ascontiguousarray(yq.transpose(2, 3, 0, 1, 4)[:, 2:])

    in_maps = []
    for c in range(N_CORES):
        sl = slice(c * ROWS, (c + 1) * ROWS)
        xs8 = (z[sl] * np.float32(-QSCALE)).astype(e4)  # [ROWS, D]
        # xq[j, s, p, h, r'] = xs8[h*512+r', j*256+s*128+p]
        xq = xs8.T.reshape(2, 2, P, 2, ROWS // 2)
        xa = np.empty((P, 2, 128 + NB), dtype=e4)
        xa[:, :, 0:128] = xq[0, :, :, 0, 0:128].transpose(1, 0, 2)
        xa[:, :, 128:] = yq[0, :, :, 0].transpose(1, 0, 2)
        xb = np.empty((P, 2, NB + 128), dtype=e4)
        xb[:, :, 0:NB] = yq[0, :, :, 1].transpose(1, 0, 2)
        xb[:, :, NB:] = xq[1, :, :, 0, 0:128].transpose(1, 0, 2)
        xc = np.ascontiguousarray(yq[1, :, :, 0].transpose(1, 0, 2))
        xd = np.ascontiguousarray(yq[1, :, :, 1].transpose(1, 0, 2))
        xe = np.empty((P, 2, 2 * 384), dtype=e4)
        xe[:, :, 0:384] = xq[0, :, :, 0, 128:512].transpose(1, 0, 2)
        xe[:, :, 384:] = xq[1, :, :, 0, 128:512].transpose(1, 0, 2)
        xf = np.empty((P, 2, 2 * NB), dtype=e4)
        xf[:, :, 0:NB] = xq[0, :, :, 1].transpose(1, 0, 2)
        xf[:, :, NB:] = xq[1, :, :, 1].transpose(1, 0, 2)
        ab_c = np.ascontiguousarray(
            ((a[sl] - 1.0) * OSCALE).astype(np.float32).reshape(M_TILES, P).T
        )  # [P, M_TILES]
        in_maps.append(
            {
                "xa": xa,
                "xb": xb,
                "xc": xc,
                "xd": xd,
                "xe": xe,
                "xf": xf,
                "pbulk": pbulk,
                "ab": ab_c,
            }
        )
    return in_maps, b


def _finish_fp8(res, b):
    q = np.concatenate([r["out"] for r in res.results], axis=0)  # int8 [N_Q, N_P]
    full = q.astype(np.float32)
    full *= np.float32(1.0 / OSCALE)
    full += (b + 1.0).astype(np.float32)[None, :]
    return full


# ---------------------------------------------------------------------------
# bf16 fallback path (previous baseline implementation)
# ---------------------------------------------------------------------------


def _build_nc_bf16(compute_dt: str):
    import concourse.mybir as mybir
    import concourse.tile as tile
    from concourse import bacc

    if compute_dt == "bf16":
        in_dt = mybir.dt.bfloat16
        mm_cast = lambda ap: ap
    elif compute_dt == "f32r":
        in_dt = mybir.dt.float32
        mm_cast = lambda ap: ap.bitcast(mybir.dt.float32r)
    else:
        raise ValueError(compute_dt)

    f32 = mybir.dt.float32
    add = mybir.AluOpType.add

    K_TILES = D // P  # 4
    WAVE_NB = NB

    nc = bacc.Bacc("TRN2", target_bir_lowering=False, debug=False, num_devices=N_CORES)

    qp = nc.dram_tensor("qp", (D, ROWS + N_P), in_dt, kind="ExternalInput")
    ab = nc.dram_tensor("ab", (P, M_TILES), f32, kind="ExternalInput")
    bb = nc.dram_tensor("bb", (1, N_P), f32, kind="ExternalInput")
    out = nc.dram_tensor("out", (ROWS, N_P), f32, kind="ExternalOutput")
    N_FRONT = ROWS + WAVE_NB  # 1536
    N_REST = N_P - 2 * WAVE_NB  # 3072

    with tile.TileContext(nc) as tc:
        with (
            tc.tile_pool(name="inputs", bufs=1) as in_pool,
            tc.tile_pool(name="outs", bufs=8) as out_pool,
            tc.tile_pool(name="psum", bufs=8, space="PSUM") as psum_pool,
        ):
            qt_tiles = [None] * K_TILES
            ptb = [[None] * K_TILES for _ in range(NBLK)]

            def load_front(k):
                fr_t = in_pool.tile([P, N_FRONT], in_dt, name=f"front_{k}")
                nc.sync.dma_start(out=fr_t, in_=qp[k * P : (k + 1) * P, 0:N_FRONT])
                qt_tiles[k] = fr_t[:, 0:ROWS]
                ptb[0][k] = fr_t[:, ROWS:N_FRONT]

            def load_b1(k):
                b1_t = in_pool.tile([P, WAVE_NB], in_dt, name=f"b1_{k}")
                nc.sync.dma_start(
                    out=b1_t, in_=qp[k * P : (k + 1) * P, N_FRONT : N_FRONT + WAVE_NB]
                )
                ptb[1][k] = b1_t

            def load_rest(k):
                re_t = in_pool.tile([P, N_REST], in_dt, name=f"rest_{k}")
                nc.sync.dma_start(
                    out=re_t,
                    in_=qp[k * P : (k + 1) * P, N_FRONT + WAVE_NB : ROWS + N_P],
                )
                for b in range(2, NBLK):
                    ptb[b][k] = re_t[:, (b - 2) * WAVE_NB : (b - 1) * WAVE_NB]

            load_front(0)
            brow_t = in_pool.tile([1, N_P], f32, name="brow_t")
            nc.sync.dma_start(out=brow_t, in_=bb[0:1, :])
            bb_t = in_pool.tile([P, N_P], f32, name="bb_t")
            nc.gpsimd.partition_broadcast(bb_t, brow_t)
            for k in range(1, K_TILES):
                load_front(k)
            for k in range(K_TILES):
                load_b1(k)
            ab_t = in_pool.tile([P, M_TILES], f32, name="ab_t")
            nc.sync.dma_start(out=ab_t, in_=ab[:, :])
            for k in range(K_TILES):
                load_rest(k)

            n_out = 0

            def epilogue(psum_t, m, b):
                nonlocal n_out
                out_t = out_pool.tile([P, WAVE_NB], f32, name="out_t")
                nc.vector.scalar_tensor_tensor(
                    out=out_t,
                    in0=psum_t,
                    scalar=ab_t[:, m : m + 1],
                    in1=bb_t[:, b * WAVE_NB : (b + 1) * WAVE_NB],
                    op0=add,
                    op1=add,
                )
                out_eng = nc.scalar if n_out % 2 == 0 else nc.sync
                n_out += 1
                out_eng.dma_start(
                    out=out[m * P : (m + 1) * P, b * WAVE_NB : (b + 1) * WAVE_NB],
                    in_=out_t,
                )

            def mm(psum_t, m, b, k):
                nc.tensor.matmul(
                    psum_t,
                    mm_cast(qt_tiles[k][:, m * P : (m + 1) * P]),
                    mm_cast(ptb[b][k]),
                    start=(k == 0),
                    stop=(k == K_TILES - 1),
                )

            psums = [
                psum_pool.tile([P, WAVE_NB], f32, name="ps", tag="ps")
                for _ in range(M_TILES)
            ]
            for k in range(K_TILES):
                for m in range(M_TILES):
                    mm(psums[m], m, 0, k)
            for m in range(M_TILES):
                epilogue(psums[m], m, 0)

            for b in range(1, NBLK):
                for m in range(M_TILES):
                    psum_t = psum_pool.tile([P, WAVE_NB], f32, name="ps", tag="ps")
                    for k in range(K_TILES):
                        mm(psum_t, m, b, k)
                    epilogue(psum_t, m, b)

    nc.compile()
    return nc


def _prep_inputs_bf16(z_queries, class_prototypes, compute_dt):
    import ml_dtypes

    np_in = ml_dtypes.bfloat16 if compute_dt == "bf16" else np.float32

    z = np.ascontiguousarray(z_queries, dtype=np.float32)
    p = np.ascontiguousarray(class_prototypes, dtype=np.float32)

    a = (z.astype(np.float64) ** 2).sum(axis=1) / D
    b = (p.astype(np.float64) ** 2).sum(axis=1) / D

    pt = (p.T * np.float32(-2.0 / D)).astype(np_in)
    bbv = np.ascontiguousarray(b.astype(np.float32).reshape(1, N_P))

    in_maps = []
    for c in range(N_CORES):
        sl = slice(c * ROWS, (c + 1) * ROWS)
        qt_c = z[sl].T.astype(np_in)
        qp_c = np.ascontiguousarray(np.concatenate([qt_c, pt], axis=1))
        ab_c = np.ascontiguousarray(
            a[sl].astype(np.float32).reshape(M_TILES, P).T
        )
        in_maps.append({"qp": qp_c, "ab": ab_c, "bb": bbv})
    return in_maps


def _get_nc(compute_dt: str):
    if compute_dt not in _CACHE:
        if compute_dt == "fp8":
            _CACHE[compute_dt] = _build_nc_fp8()
        else:
            _CACHE[compute_dt] = _build_nc_bf16(compute_dt)
    return _CACHE[compute_dt]


def run(z_queries, class_prototypes, compute_dt=COMPUTE_DT, **spmd_kwargs):
    from concourse.bass_utils import run_bass_kernel_spmd

    nc = _get_nc(compute_dt)
    if compute_dt == "fp8":
        in_maps, b = _prep_inputs_fp8(z_queries, class_prototypes)
        res = run_bass_kernel_spmd(
            nc, in_maps, core_ids=list(range(N_CORES)), **spmd_kwargs
        )
        full = _finish_fp8(res, b)
    else:
        in_maps = _prep_inputs_bf16(z_queries, class_prototypes, compute_dt)
        res = run_bass_kernel_spmd(
            nc, in_maps, core_ids=list(range(N_CORES)), **spmd_kwargs
        )
        full = np.concatenate([r["out"] for r in res.results], axis=0)
    return full, res


def kernel(z_queries: np.ndarray, class_prototypes: np.ndarray) -> np.ndarray:
    full, _ = run(z_queries, class_prototypes)
    return full


# revision 20
# speedup vs baseline: 1.0430x; 1.0430x over previous
"""Pairwise squared-distance kernel for Trainium2 (8 NeuronCores).

out[i, j] = mean_d (x_i[d] - y_j[d])^2
          = (||x_i||^2 + ||y_j||^2 - 2 x_i . y_j) / D

Sharding: rows of z_queries split across 8 cores (1024 rows each);
class_prototypes replicated. Each core computes its [1024, 4096] slab.

fp8 device kernel (per core), COMPUTE_DT="fp8":
  - inputs quantized to fp8 e4m3 with asymmetric scales (x * -2^-4,
    y * 2^-4) so PSUM accumulates -2*x.y/D directly without pushing
    small values into the fp8 subnormal range.
  - GEMM in DoubleRow perf mode: lhsT [128,(2,128)] / rhs [128,(2,512)]
    contract K=256 per matmul; 2 matmuls per [128,512] psum half.
  - PE warm-up: a short stream of zero dummy matmuls starts the DVFS
    ramp while the first input DMA is in flight.
  - c0 split so the 160 KiB needed by the first real matmul lands first.
  - epilogue split 3-way (Act / DVE / Pool), one op per [128,1024] psum
    tile (Activation Identity: psum*S + (a_i-1)*S, or tensor_scalar
    mult/add), output int8 (S=120); norms a_i exact from host fp64.
  - output DMA'd per psum tile ([128,1024] int8) on the DVE/Pool queues
    (input owns the sync queue); host dequantizes q/S + 1 + b_j.
"""

import sys

if "/opt/trn_rl_repo" not in sys.path:
    sys.path.insert(0, "/opt/trn_rl_repo")

import numpy as np

N_CORES = 8
N_Q = 8192
N_P = 4096
D = 512
ROWS = N_Q // N_CORES  # 1024 query rows per core
P = 128
M_TILES = ROWS // P  # 8
NB = 512  # matmul moving free dim per k-slot (1 psum bank fp32)
NBLK = N_P // NB  # 8 column blocks of 512
QSCALE = 2.0**-4  # asymmetric fp8 pre-scale; product carries -2/D = -2^-8
OSCALE = 120.0  # int8 output scale: q = (psum + a_i - 1) * OSCALE

COMPUTE_DT = "fp8"

N_DUMMY = 22  # PE warm-up matmuls bridging until the first two input pieces land

_CACHE = {}


def _build_nc_fp8():
    import concourse.mybir as mybir
    import concourse.tile as tile
    from concourse import bacc

    fp8 = mybir.dt.float8e4
    f32 = mybir.dt.float32
    i8 = mybir.dt.int8
    DR = mybir.MatmulPerfMode.DoubleRow

    nc = bacc.Bacc("TRN2", target_bir_lowering=False, debug=False, num_devices=N_CORES)

    # DoubleRow k-packing: k = j*256 + s*128 + p  (j: double-tile, s: slot)
    # Input in three consumption-ordered pieces (few pieces: each DMA has
    # ~0.25us of queue dead time, so merging beats fine-grained gating):
    #   xa   = [q j0 h0 m0 | p b0 j0]                       (first matmul)
    #   xbcd = [p b1 j0 | q j1 h0 m0 | p b0 j1 | p b1 j1]   (rest of tile 0)
    #   xef  = [q j0 h0 m1-3 | q j1 h0 m1-3 | q j0 h1 | q j1 h1]
    xa = nc.dram_tensor("xa", (P, 2, 128 + NB), fp8, kind="ExternalInput")
    xbcd = nc.dram_tensor("xbcd", (P, 2, 3 * NB + 128), fp8, kind="ExternalInput")
    xef = nc.dram_tensor("xef", (P, 2, 2 * 384 + 2 * NB), fp8, kind="ExternalInput")
    pbulk = nc.dram_tensor("pbulk", (P, 6, 2, 2, NB), fp8, kind="ExternalInput")
    ab = nc.dram_tensor("ab", (P, M_TILES), f32, kind="ExternalInput")  # (a-1)*S
    out = nc.dram_tensor("out", (ROWS, N_P), i8, kind="ExternalOutput")

    with tile.TileContext(nc) as tc:
        with (
            tc.tile_pool(name="inputs", bufs=1) as in_pool,
            tc.tile_pool(name="outs", bufs=8) as out_pool,
            tc.tile_pool(name="psum", bufs=4, space="PSUM") as psum_pool,
        ):
            pt = [None, None]  # bulk tiles: [b2,b3], [b4..b7]

            # dummy operands for the PE warm-up stream (zeros)
            dw = in_pool.tile([P, 2, 128], fp8, name="dw")
            nc.vector.memset(dw, 0.0)
            dm = in_pool.tile([P, 2, 128], fp8, name="dm")
            nc.vector.memset(dm, 0.0)

            # preload the Scalar engine's activation table while idle so the
            # first real Identity epilogue doesn't eat the 1.3us table load
            warm_t = in_pool.tile([P, 1], f32, name="warm")
            nc.vector.memset(warm_t, 0.0)
            warm_o = in_pool.tile([P, 1], f32, name="warm_o")
            nc.scalar.activation(
                warm_o, warm_t, func=mybir.ActivationFunctionType.Identity
            )

            # input stream: strictly ordered on the sync queue (q1); ab on q10
            ta = in_pool.tile([P, 2, 128 + NB], fp8, name="ta")
            nc.sync.dma_start(out=ta, in_=xa[:, :])
            tbcd = in_pool.tile([P, 2, 3 * NB + 128], fp8, name="tbcd")
            nc.sync.dma_start(out=tbcd, in_=xbcd[:, :])
            ab_t = in_pool.tile([P, M_TILES], f32, name="ab_t")
            nc.scalar.dma_start(out=ab_t, in_=ab[:, :])
            tef = in_pool.tile([P, 2, 2 * 384 + 2 * NB], fp8, name="tef")
            nc.sync.dma_start(out=tef, in_=xef[:, :])
            pt[0] = in_pool.tile([P, 2, 2, 2, NB], fp8, name="pb0")
            nc.sync.dma_start(out=pt[0], in_=pbulk[:, 0:2])
            pt[1] = in_pool.tile([P, 4, 2, 2, NB], fp8, name="pb1")
            nc.sync.dma_start(out=pt[1], in_=pbulk[:, 2:6])

            # PE warm-up: a gapless stream of zero matmuls bridges the Tensor
            # engine from the preamble barrier to the first real matmul so
            # the DVFS ramp starts as early as possible
            dummy_ps = psum_pool.tile([P, 2 * NB], f32, name="ps", tag="ps")
            for _ in range(N_DUMMY):
                nc.tensor.matmul(
                    dummy_ps[:, 0:128], dw, dm, start=True, stop=True, perf_mode=DR
                )

            def rhs_ap(b, j):
                if b == 0:
                    return (
                        ta[:, :, 128 : 128 + NB]
                        if j == 0
                        else tbcd[:, :, NB + 128 : 2 * NB + 128]
                    )
                if b == 1:
                    return (
                        tbcd[:, :, 0:NB]
                        if j == 0
                        else tbcd[:, :, 2 * NB + 128 : 3 * NB + 128]
                    )
                if b < 4:
                    return pt[0][:, b - 2, j]
                return pt[1][:, b - 4, j]

            def lhs_ap(j, m):
                h, mm_ = divmod(m, 4)
                if h == 0:
                    if mm_ == 0:
                        return (
                            ta[:, :, 0:128]
                            if j == 0
                            else tbcd[:, :, NB : NB + 128]
                        )
                    off = 0 if j == 0 else 384
                    return tef[:, :, off + (mm_ - 1) * 128 : off + mm_ * 128]
                off = 768 + (0 if j == 0 else NB)
                return tef[:, :, off + mm_ * P : off + (mm_ + 1) * P]

            def epi_act(dst, ps, m):
                nc.scalar.activation(
                    dst,
                    ps,
                    func=mybir.ActivationFunctionType.Identity,
                    bias=ab_t[:, m : m + 1],
                    scale=float(OSCALE),
                )

            def _epi_ts(eng, dst, ps, m):
                eng.tensor_scalar(
                    out=dst,
                    in0=ps,
                    scalar1=float(OSCALE),
                    scalar2=ab_t[:, m : m + 1],
                    op0=mybir.AluOpType.mult,
                    op1=mybir.AluOpType.add,
                )

            def epi_dve(dst, ps, m):
                _epi_ts(nc.vector, dst, ps, m)

            # epilogue split Act/DVE greedily (GpSimd cannot access PSUM);
            # output-DMA triggers: first 8 tiles on scalar (q10) while the
            # input stream still owns the sync queue (q1), then alternate
            # sync (q1) / gpsimd (q0, SW-DGE at ~670ns) to spread transfers
            epi_busy = {"A": 0.0, "D": 0.0}
            epi_cost = {"A": 1150.0, "D": 1290.0}
            epi_fn = {"A": epi_act, "D": epi_dve}
            trig_eng = {"A": nc.scalar, "S": nc.sync, "P": nc.gpsimd}
            N_TILES = 4 * M_TILES
            N_SPLIT = 5  # last tiles get A/D-split epilogues so queues drain
            epi_seq = []
            trig_seq = []
            for idx in range(N_TILES - N_SPLIT):
                e = min(epi_busy, key=lambda k: epi_busy[k] + epi_cost[k])
                epi_busy[e] += epi_cost[e]
                epi_seq.append(e)
                if idx < 8:
                    trig_seq.append("A")
                    epi_busy["A"] += 607.0
                else:
                    trig_seq.append("S" if idx % 2 == 0 else "P")

            gi = 0
            for p in range(4):  # column-block pair (2p, 2p+1)
                for m in range(M_TILES):
                    ps = psum_pool.tile([P, 2 * NB], f32, name="ps", tag="ps")
                    for j in (0, 1):
                        lw = lhs_ap(j, m)
                        for i in (0, 1):
                            nc.tensor.matmul(
                                ps[:, i * NB : (i + 1) * NB],
                                lw,
                                rhs_ap(2 * p + i, j),
                                start=(j == 0),
                                stop=(j == 1),
                                perf_mode=DR,
                            )
                    row = slice(m * P, (m + 1) * P)
                    col = slice(p * 2 * NB, (p + 1) * 2 * NB)
                    tile_idx = p * M_TILES + m
                    if tile_idx >= N_TILES - N_SPLIT:
                        # tail tiles: split the epilogue across both psum
                        # engines into two INDEPENDENT out tiles (a shared
                        # tile falsely serializes the two writers) so both
                        # engine queues stay drained toward the finish
                        last = tile_idx == N_TILES - 1
                        oa = out_pool.tile([P, NB], i8, name="oa", tag="oh")
                        od = out_pool.tile([P, NB], i8, name="od", tag="oh")
                        epi_dve(od, ps[:, NB:], m)
                        epi_act(oa, ps[:, 0:NB], m)
                        e1 = nc.sync if (last or tile_idx % 2 == 0) else nc.gpsimd
                        e2 = nc.scalar if last else (
                            nc.gpsimd if tile_idx % 2 == 0 else nc.sync
                        )
                        e1.dma_start(
                            out=out[row, p * 2 * NB + NB : (p + 1) * 2 * NB],
                            in_=od,
                        )
                        e2.dma_start(
                            out=out[row, p * 2 * NB : p * 2 * NB + NB],
                            in_=oa,
                        )
                    else:
                        o = out_pool.tile([P, 2 * NB], i8, name="ot")
                        epi_fn[epi_seq[gi]](o, ps, m)
                        trig_eng[trig_seq[gi]].dma_start(out=out[row, col], in_=o)
                        gi += 1

    nc.compile()
    return nc


def _prep_inputs_fp8(z_queries, class_prototypes):
    import ml_dtypes

    e4 = ml_dtypes.float8_e4m3

    z = np.ascontiguousarray(z_queries, dtype=np.float32)
    pr = np.ascontiguousarray(class_prototypes, dtype=np.float32)

    a = (z.astype(np.float64) ** 2).sum(axis=1) / D  # (N_Q,) ||x||^2 / D
    b = (pr.astype(np.float64) ** 2).sum(axis=1) / D  # (N_P,) ||y||^2 / D

    ys8 = (pr * np.float32(QSCALE)).astype(e4)  # [N_P, D]
    # yq[j, s, p, b, c] = ys8[b*512+c, j*256+s*128+p]
    yq = ys8.T.reshape(2, 2, P, NBLK, NB)
    # pbulk[p, b-2, j, s, c]
    pbulk = np.ascontiguousarray(yq.transpose(2, 3, 0, 1, 4)[:, 2:])

    in_maps = []
    for c in range(N_CORES):
        sl = slice(c * ROWS, (c + 1) * ROWS)
        xs8 = (z[sl] * np.float32(-QSCALE)).astype(e4)  # [ROWS, D]
        # xq[j, s, p, h, r'] = xs8[h*512+r', j*256+s*128+p]
        xq = xs8.T.reshape(2, 2, P, 2, ROWS // 2)
        xa = np.empty((P, 2, 128 + NB), dtype=e4)
        xa[:, :, 0:128] = xq[0, :, :, 0, 0:128].transpose(1, 0, 2)
        xa[:, :, 128:] = yq[0, :, :, 0].transpose(1, 0, 2)
        xbcd = np.empty((P, 2, 3 * NB + 128), dtype=e4)
        xbcd[:, :, 0:NB] = yq[0, :, :, 1].transpose(1, 0, 2)
        xbcd[:, :, NB : NB + 128] = xq[1, :, :, 0, 0:128].transpose(1, 0, 2)
        xbcd[:, :, NB + 128 : 2 * NB + 128] = yq[1, :, :, 0].transpose(1, 0, 2)
        xbcd[:, :, 2 * NB + 128 :] = yq[1, :, :, 1].transpose(1, 0, 2)
        xef = np.empty((P, 2, 2 * 384 + 2 * NB), dtype=e4)
        xef[:, :, 0:384] = xq[0, :, :, 0, 128:512].transpose(1, 0, 2)
        xef[:, :, 384:768] = xq[1, :, :, 0, 128:512].transpose(1, 0, 2)
        xef[:, :, 768 : 768 + NB] = xq[0, :, :, 1].transpose(1, 0, 2)
        xef[:, :, 768 + NB :] = xq[1, :, :, 1].transpose(1, 0, 2)
        ab_c = np.ascontiguousarray(
            ((a[sl] - 1.0) * OSCALE).astype(np.float32).reshape(M_TILES, P).T
        )  # [P, M_TILES]
        in_maps.append(
            {"xa": xa, "xbcd": xbcd, "xef": xef, "pbulk": pbulk, "ab": ab_c}
        )
    return in_maps, b


def _finish_fp8(res, b):
    q = np.concatenate([r["out"] for r in res.results], axis=0)  # int8 [N_Q, N_P]
    full = q.astype(np.float32)
    full *= np.float32(1.0 / OSCALE)
    full += (b + 1.0).astype(np.float32)[None, :]
    return full


# ---------------------------------------------------------------------------
# bf16 fallback path (previous baseline implementation)
# ---------------------------------------------------------------------------


def _build_nc_bf16(compute_dt: str):
    import concourse.mybir as mybir
    import concourse.tile as tile
    from concourse import bacc

    if compute_dt == "bf16":
        in_dt = mybir.dt.bfloat16
        mm_cast = lambda ap: ap
    elif compute_dt == "f32r":
        in_dt = mybir.dt.float32
        mm_cast = lambda ap: ap.bitcast(mybir.dt.float32r)
    else:
        raise ValueError(compute_dt)

    f32 = mybir.dt.float32
    add = mybir.AluOpType.add

    K_TILES = D // P  # 4
    WAVE_NB = NB

    nc = bacc.Bacc("TRN2", target_bir_lowering=False, debug=False, num_devices=N_CORES)

    qp = nc.dram_tensor("qp", (D, ROWS + N_P), in_dt, kind="ExternalInput")
    ab = nc.dram_tensor("ab", (P, M_TILES), f32, kind="ExternalInput")
    bb = nc.dram_tensor("bb", (1, N_P), f32, kind="ExternalInput")
    out = nc.dram_tensor("out", (ROWS, N_P), f32, kind="ExternalOutput")
    N_FRONT = ROWS + WAVE_NB  # 1536
    N_REST = N_P - 2 * WAVE_NB  # 3072

    with tile.TileContext(nc) as tc:
        with (
            tc.tile_pool(name="inputs", bufs=1) as in_pool,
            tc.tile_pool(name="outs", bufs=8) as out_pool,
            tc.tile_pool(name="psum", bufs=8, space="PSUM") as psum_pool,
        ):
            qt_tiles = [None] * K_TILES
            ptb = [[None] * K_TILES for _ in range(NBLK)]

            def load_front(k):
                fr_t = in_pool.tile([P, N_FRONT], in_dt, name=f"front_{k}")
                nc.sync.dma_start(out=fr_t, in_=qp[k * P : (k + 1) * P, 0:N_FRONT])
                qt_tiles[k] = fr_t[:, 0:ROWS]
                ptb[0][k] = fr_t[:, ROWS:N_FRONT]

            def load_b1(k):
                b1_t = in_pool.tile([P, WAVE_NB], in_dt, name=f"b1_{k}")
                nc.sync.dma_start(
                    out=b1_t, in_=qp[k * P : (k + 1) * P, N_FRONT : N_FRONT + WAVE_NB]
                )
                ptb[1][k] = b1_t

            def load_rest(k):
                re_t = in_pool.tile([P, N_REST], in_dt, name=f"rest_{k}")
                nc.sync.dma_start(
                    out=re_t,
                    in_=qp[k * P : (k + 1) * P, N_FRONT + WAVE_NB : ROWS + N_P],
                )
                for b in range(2, NBLK):
                    ptb[b][k] = re_t[:, (b - 2) * WAVE_NB : (b - 1) * WAVE_NB]

            load_front(0)
            brow_t = in_pool.tile([1, N_P], f32, name="brow_t")
            nc.sync.dma_start(out=brow_t, in_=bb[0:1, :])
            bb_t = in_pool.tile([P, N_P], f32, name="bb_t")
            nc.gpsimd.partition_broadcast(bb_t, brow_t)
            for k in range(1, K_TILES):
                load_front(k)
            for k in range(K_TILES):
                load_b1(k)
            ab_t = in_pool.tile([P, M_TILES], f32, name="ab_t")
            nc.sync.dma_start(out=ab_t, in_=ab[:, :])
            for k in range(K_TILES):
                load_rest(k)

            n_out = 0

            def epilogue(psum_t, m, b):
                nonlocal n_out
                out_t = out_pool.tile([P, WAVE_NB], f32, name="out_t")
                nc.vector.scalar_tensor_tensor(
                    out=out_t,
                    in0=psum_t,
                    scalar=ab_t[:, m : m + 1],
                    in1=bb_t[:, b * WAVE_NB : (b + 1) * WAVE_NB],
                    op0=add,
                    op1=add,
                )
                out_eng = nc.scalar if n_out % 2 == 0 else nc.sync
                n_out += 1
                out_eng.dma_start(
                    out=out[m * P : (m + 1) * P, b * WAVE_NB : (b + 1) * WAVE_NB],
                    in_=out_t,
                )

            def mm(psum_t, m, b, k):
                nc.tensor.matmul(
                    psum_t,
                    mm_cast(qt_tiles[k][:, m * P : (m + 1) * P]),
                    mm_cast(ptb[b][k]),
                    start=(k == 0),
                    stop=(k == K_TILES - 1),
                )

            psums = [
                psum_pool.tile([P, WAVE_NB], f32, name="ps", tag="ps")
                for _ in range(M_TILES)
            ]
            for k in range(K_TILES):
                for m in range(M_TILES):
                    mm(psums[m], m, 0, k)
            for m in range(M_TILES):
                epilogue(psums[m], m, 0)

            for b in range(1, NBLK):
                for m in range(M_TILES):
                    psum_t = psum_pool.tile([P, WAVE_NB], f32, name="ps", tag="ps")
                    for k in range(K_TILES):
                        mm(psum_t, m, b, k)
                    epilogue(psum_t, m, b)

    nc.compile()
    return nc


def _prep_inputs_bf16(z_queries, class_prototypes, compute_dt):
    import ml_dtypes

    np_in = ml_dtypes.bfloat16 if compute_dt == "bf16" else np.float32

    z = np.ascontiguousarray(z_queries, dtype=np.float32)
    p = np.ascontiguousarray(class_prototypes, dtype=np.float32)

    a = (z.astype(np.float64) ** 2).sum(axis=1) / D
    b = (p.astype(np.float64) ** 2).sum(axis=1) / D

    pt = (p.T * np.float32(-2.0 / D)).astype(np_in)
    bbv = np.ascontiguousarray(b.astype(np.float32).reshape(1, N_P))

    in_maps = []
    for c in range(N_CORES):
        sl = slice(c * ROWS, (c + 1) * ROWS)
        qt_c = z[sl].T.astype(np_in)
        qp_c = np.ascontiguousarray(np.concatenate([qt_c, pt], axis=1))
        ab_c = np.ascontiguousarray(
            a[sl].astype(np.float32).reshape(M_TILES, P).T
        )
        in_maps.append({"qp": qp_c, "ab": ab_c, "bb": bbv})
    return in_maps


def _get_nc(compute_dt: str):
    if compute_dt not in _CACHE:
        if compute_dt == "fp8":
            _CACHE[compute_dt] = _build_nc_fp8()
        else:
            _CACHE[compute_dt] = _build_nc_bf16(compute_dt)
    return _CACHE[compute_dt]


def run(z_queries, class_prototypes, compute_dt=COMPUTE_DT, **spmd_kwargs):
    from concourse.bass_utils import run_bass_kernel_spmd

    nc = _get_nc(compute_dt)
    if compute_dt == "fp8":
        in_maps, b = _prep_inputs_fp8(z_queries, class_prototypes)
        res = run_bass_kernel_spmd(
            nc, in_maps, core_ids=list(range(N_CORES)), **spmd_kwargs
        )
        full = _finish_fp8(res, b)
    else:
        in_maps = _prep_inputs_bf16(z_queries, class_prototypes, compute_dt)
        res = run_bass_kernel_spmd(
            nc, in_maps, core_ids=list(range(N_CORES)), **spmd_kwargs
        )
        full = np.concatenate([r["out"] for r in res.results], axis=0)
    return full, res


def kernel(z_queries: np.ndarray, class_prototypes: np.ndarray) -> np.ndarray:
    full, _ = run(z_queries, class_prototypes)
    return full


# revision 23
# speedup vs baseline: 1.0554x; 1.0118x over previous
"""Pairwise squared-distance kernel for Trainium2 (8 NeuronCores).

out[i, j] = mean_d (x_i[d] - y_j[d])^2
          = (||x_i||^2 + ||y_j||^2 - 2 x_i . y_j) / D

Sharding: rows of z_queries split across 8 cores (1024 rows each);
class_prototypes replicated. Each core computes its [1024, 4096] slab.

fp8 device kernel (per core), COMPUTE_DT="fp8":
  - inputs quantized to fp8 e4m3 with asymmetric scales (x * -2^-4,
    y * 2^-4) so PSUM accumulates -2*x.y/D directly without pushing
    small values into the fp8 subnormal range.
  - GEMM in DoubleRow perf mode: lhsT [128,(2,128)] / rhs [128,(2,512)]
    contract K=256 per matmul; 2 matmuls per [128,512] psum half.
  - PE warm-up: a short stream of zero dummy matmuls starts the DVFS
    ramp while the first input DMA is in flight.
  - c0 split so the 160 KiB needed by the first real matmul lands first.
  - epilogue split 3-way (Act / DVE / Pool), one op per [128,1024] psum
    tile (Activation Identity: psum*S + (a_i-1)*S, or tensor_scalar
    mult/add), output int8 (S=120); norms a_i exact from host fp64.
  - output DMA'd per psum tile ([128,1024] int8) on the DVE/Pool queues
    (input owns the sync queue); host dequantizes q/S + 1 + b_j.
"""

import sys

if "/opt/trn_rl_repo" not in sys.path:
    sys.path.insert(0, "/opt/trn_rl_repo")

import numpy as np

N_CORES = 8
N_Q = 8192
N_P = 4096
D = 512
ROWS = N_Q // N_CORES  # 1024 query rows per core
P = 128
M_TILES = ROWS // P  # 8
NB = 512  # matmul moving free dim per k-slot (1 psum bank fp32)
NBLK = N_P // NB  # 8 column blocks of 512
QSCALE = 2.0**-4  # asymmetric fp8 pre-scale; product carries -2/D = -2^-8
OSCALE = 120.0  # int8 output scale: q = (psum + a_i - 1) * OSCALE

COMPUTE_DT = "fp8"

N_DUMMY = 18  # PE warm-up matmuls bridging until the first two input pieces land

_CACHE = {}


def _build_nc_fp8():
    import concourse.mybir as mybir
    import concourse.tile as tile
    from concourse import bacc

    fp8 = mybir.dt.float8e4
    f32 = mybir.dt.float32
    i8 = mybir.dt.int8
    DR = mybir.MatmulPerfMode.DoubleRow

    nc = bacc.Bacc("TRN2", target_bir_lowering=False, debug=False, num_devices=N_CORES)

    # DoubleRow k-packing: k = j*256 + s*128 + p  (j: double-tile, s: slot)
    # Input in three consumption-ordered pieces (few pieces: each DMA has
    # ~0.25us of queue dead time, so merging beats fine-grained gating):
    #   xa   = [q j0 h0 m0 | p b0 j0]                       (first matmul)
    #   xbcd = [p b1 j0 | q j1 h0 m0 | p b0 j1 | p b1 j1]   (rest of tile 0)
    #   xef  = [q j0 h0 m1-3 | q j1 h0 m1-3 | q j0 h1 | q j1 h1]
    xa = nc.dram_tensor("xa", (P, 2, 128 + NB), fp8, kind="ExternalInput")
    xbcd = nc.dram_tensor("xbcd", (P, 2, 3 * NB + 128), fp8, kind="ExternalInput")
    xef = nc.dram_tensor("xef", (P, 2, 2 * 384 + 2 * NB), fp8, kind="ExternalInput")
    pbulk = nc.dram_tensor("pbulk", (P, 6, 2, 2, NB), fp8, kind="ExternalInput")
    ab = nc.dram_tensor("ab", (P, M_TILES), f32, kind="ExternalInput")  # (a-1)*S
    out = nc.dram_tensor("out", (ROWS, N_P), i8, kind="ExternalOutput")

    with tile.TileContext(nc) as tc:
        with (
            tc.tile_pool(name="inputs", bufs=1) as in_pool,
            tc.tile_pool(name="outs", bufs=8) as out_pool,
            tc.tile_pool(name="psum", bufs=4, space="PSUM") as psum_pool,
        ):
            pt = [None, None]  # bulk tiles: [b2,b3], [b4..b7]

            # dummy operands for the PE warm-up stream (zeros)
            dw = in_pool.tile([P, 2, 128], fp8, name="dw")
            nc.vector.memset(dw, 0.0)
            dm = in_pool.tile([P, 2, 128], fp8, name="dm")
            nc.vector.memset(dm, 0.0)

            # input stream on two parallel queues:
            #   q1 (sync):    ta, tef, pt0, pt1
            #   q10 (scalar): tbcd first (needed by matmul 2-4), then ab,
            #                 then the warm-up activation-table load
            ta = in_pool.tile([P, 2, 128 + NB], fp8, name="ta")
            nc.sync.dma_start(out=ta, in_=xa[:, :])
            tbcd = in_pool.tile([P, 2, 3 * NB + 128], fp8, name="tbcd")
            nc.scalar.dma_start(out=tbcd, in_=xbcd[:, :])
            tef = in_pool.tile([P, 2, 2 * 384 + 2 * NB], fp8, name="tef")
            nc.sync.dma_start(out=tef, in_=xef[:, :])
            ab_t = in_pool.tile([P, M_TILES], f32, name="ab_t")
            nc.scalar.dma_start(out=ab_t, in_=ab[:, :])
            pt[0] = in_pool.tile([P, 2, 2, 2, NB], fp8, name="pb0")
            nc.sync.dma_start(out=pt[0], in_=pbulk[:, 0:2])
            pt[1] = in_pool.tile([P, 4, 2, 2, NB], fp8, name="pb1")
            nc.sync.dma_start(out=pt[1], in_=pbulk[:, 2:6])

            # preload the Scalar engine's activation table while idle so the
            # first real Identity epilogue doesn't eat the 1.3us table load
            warm_t = in_pool.tile([P, 1], f32, name="warm")
            nc.vector.memset(warm_t, 0.0)
            warm_o = in_pool.tile([P, 1], f32, name="warm_o")
            nc.scalar.activation(
                warm_o, warm_t, func=mybir.ActivationFunctionType.Identity
            )

            # PE warm-up: a gapless stream of zero matmuls bridges the Tensor
            # engine from the preamble barrier to the first real matmul so
            # the DVFS ramp starts as early as possible
            dummy_ps = psum_pool.tile([P, 2 * NB], f32, name="ps", tag="ps")
            for _ in range(N_DUMMY):
                nc.tensor.matmul(
                    dummy_ps[:, 0:128], dw, dm, start=True, stop=True, perf_mode=DR
                )

            def rhs_ap(b, j):
                if b == 0:
                    return (
                        ta[:, :, 128 : 128 + NB]
                        if j == 0
                        else tbcd[:, :, NB + 128 : 2 * NB + 128]
                    )
                if b == 1:
                    return (
                        tbcd[:, :, 0:NB]
                        if j == 0
                        else tbcd[:, :, 2 * NB + 128 : 3 * NB + 128]
                    )
                if b < 4:
                    return pt[0][:, b - 2, j]
                return pt[1][:, b - 4, j]

            def lhs_ap(j, m):
                h, mm_ = divmod(m, 4)
                if h == 0:
                    if mm_ == 0:
                        return (
                            ta[:, :, 0:128]
                            if j == 0
                            else tbcd[:, :, NB : NB + 128]
                        )
                    off = 0 if j == 0 else 384
                    return tef[:, :, off + (mm_ - 1) * 128 : off + mm_ * 128]
                off = 768 + (0 if j == 0 else NB)
                return tef[:, :, off + mm_ * P : off + (mm_ + 1) * P]

            def epi_act(dst, ps, m):
                nc.scalar.activation(
                    dst,
                    ps,
                    func=mybir.ActivationFunctionType.Identity,
                    bias=ab_t[:, m : m + 1],
                    scale=float(OSCALE),
                )

            def _epi_ts(eng, dst, ps, m):
                eng.tensor_scalar(
                    out=dst,
                    in0=ps,
                    scalar1=float(OSCALE),
                    scalar2=ab_t[:, m : m + 1],
                    op0=mybir.AluOpType.mult,
                    op1=mybir.AluOpType.add,
                )

            def epi_dve(dst, ps, m):
                _epi_ts(nc.vector, dst, ps, m)

            # epilogue split Act/DVE greedily (GpSimd cannot access PSUM);
            # output-DMA triggers: first 8 tiles on scalar (q10) while the
            # input stream still owns the sync queue (q1), then alternate
            # sync (q1) / gpsimd (q0, SW-DGE at ~670ns) to spread transfers
            epi_busy = {"A": 0.0, "D": 0.0}
            epi_cost = {"A": 1150.0, "D": 1290.0}
            epi_fn = {"A": epi_act, "D": epi_dve}
            trig_eng = {"A": nc.scalar, "S": nc.sync, "P": nc.gpsimd}
            N_TILES = 4 * M_TILES
            N_SPLIT = 9  # last tiles get A/D-split epilogues so queues drain
            epi_seq = []
            trig_seq = []
            for idx in range(N_TILES - N_SPLIT):
                e = min(epi_busy, key=lambda k: epi_busy[k] + epi_cost[k])
                epi_busy[e] += epi_cost[e]
                epi_seq.append(e)
                if idx < 8:
                    trig_seq.append("A")
                    epi_busy["A"] += 607.0
                else:
                    trig_seq.append("S" if idx % 2 == 0 else "P")

            gi = 0
            for p in range(4):  # column-block pair (2p, 2p+1)
                for m in range(M_TILES):
                    ps = psum_pool.tile([P, 2 * NB], f32, name="ps", tag="ps")
                    for j in (0, 1):
                        lw = lhs_ap(j, m)
                        for i in (0, 1):
                            nc.tensor.matmul(
                                ps[:, i * NB : (i + 1) * NB],
                                lw,
                                rhs_ap(2 * p + i, j),
                                start=(j == 0),
                                stop=(j == 1),
                                perf_mode=DR,
                            )
                    row = slice(m * P, (m + 1) * P)
                    col = slice(p * 2 * NB, (p + 1) * 2 * NB)
                    tile_idx = p * M_TILES + m
                    if tile_idx >= N_TILES - N_SPLIT:
                        # tail tiles: split the epilogue across both psum
                        # engines into two INDEPENDENT out tiles (a shared
                        # tile falsely serializes the two writers) so both
                        # engine queues stay drained toward the finish
                        last = tile_idx == N_TILES - 1
                        oa = out_pool.tile([P, NB], i8, name="oa", tag="oh")
                        od = out_pool.tile([P, NB], i8, name="od", tag="oh")
                        epi_dve(od, ps[:, NB:], m)
                        epi_act(oa, ps[:, 0:NB], m)
                        e1 = nc.sync if (last or tile_idx % 2 == 0) else nc.gpsimd
                        e2 = nc.scalar if last else (
                            nc.gpsimd if tile_idx % 2 == 0 else nc.sync
                        )
                        e1.dma_start(
                            out=out[row, p * 2 * NB + NB : (p + 1) * 2 * NB],
                            in_=od,
                        )
                        e2.dma_start(
                            out=out[row, p * 2 * NB : p * 2 * NB + NB],
                            in_=oa,
                        )
                    else:
                        o = out_pool.tile([P, 2 * NB], i8, name="ot")
                        epi_fn[epi_seq[gi]](o, ps, m)
                        trig_eng[trig_seq[gi]].dma_start(out=out[row, col], in_=o)
                        gi += 1

    nc.compile()
    return nc


def _prep_inputs_fp8(z_queries, class_prototypes):
    import ml_dtypes

    e4 = ml_dtypes.float8_e4m3

    z = np.ascontiguousarray(z_queries, dtype=np.float32)
    pr = np.ascontiguousarray(class_prototypes, dtype=np.float32)

    a = (z.astype(np.float64) ** 2).sum(axis=1) / D  # (N_Q,) ||x||^2 / D
    b = (pr.astype(np.float64) ** 2).sum(axis=1) / D  # (N_P,) ||y||^2 / D

    ys8 = (pr * np.float32(QSCALE)).astype(e4)  # [N_P, D]
    # yq[j, s, p, b, c] = ys8[b*512+c, j*256+s*128+p]
    yq = ys8.T.reshape(2, 2, P, NBLK, NB)
    # pbulk[p, b-2, j, s, c]
    pbulk = np.ascontiguousarray(yq.transpose(2, 3, 0, 1, 4)[:, 2:])

    in_maps = []
    for c in range(N_CORES):
        sl = slice(c * ROWS, (c + 1) * ROWS)
        xs8 = (z[sl] * np.float32(-QSCALE)).astype(e4)  # [ROWS, D]
        # xq[j, s, p, h, r'] = xs8[h*512+r', j*256+s*128+p]
        xq = xs8.T.reshape(2, 2, P, 2, ROWS // 2)
        xa = np.empty((P, 2, 128 + NB), dtype=e4)
        xa[:, :, 0:128] = xq[0, :, :, 0, 0:128].transpose(1, 0, 2)
        xa[:, :, 128:] = yq[0, :, :, 0].transpose(1, 0, 2)
        xbcd = np.empty((P, 2, 3 * NB + 128), dtype=e4)
        xbcd[:, :, 0:NB] = yq[0, :, :, 1].transpose(1, 0, 2)
        xbcd[:, :, NB : NB + 128] = xq[1, :, :, 0, 0:128].transpose(1, 0, 2)
        xbcd[:, :, NB + 128 : 2 * NB + 128] = yq[1, :, :, 0].transpose(1, 0, 2)
        xbcd[:, :, 2 * NB + 128 :] = yq[1, :, :, 1].transpose(1, 0, 2)
        xef = np.empty((P, 2, 2 * 384 + 2 * NB), dtype=e4)
        xef[:, :, 0:384] = xq[0, :, :, 0, 128:512].transpose(1, 0, 2)
        xef[:, :, 384:768] = xq[1, :, :, 0, 128:512].transpose(1, 0, 2)
        xef[:, :, 768 : 768 + NB] = xq[0, :, :, 1].transpose(1, 0, 2)
        xef[:, :, 768 + NB :] = xq[1, :, :, 1].transpose(1, 0, 2)
        ab_c = np.ascontiguousarray(
            ((a[sl] - 1.0) * OSCALE).astype(np.float32).reshape(M_TILES, P).T
        )  # [P, M_TILES]
        in_maps.append(
            {"xa": xa, "xbcd": xbcd, "xef": xef, "pbulk": pbulk, "ab": ab_c}
        )
    return in_maps, b


def _finish_fp8(res, b):
    q = np.concatenate([r["out"] for r in res.results], axis=0)  # int8 [N_Q, N_P]
    full = q.astype(np.float32)
    full *= np.float32(1.0 / OSCALE)
    full += (b + 1.0).astype(np.float32)[None, :]
    return full


# ---------------------------------------------------------------------------
# bf16 fallback path (previous baseline implementation)
# ---------------------------------------------------------------------------


def _build_nc_bf16(compute_dt: str):
    import concourse.mybir as mybir
    import concourse.tile as tile
    from concourse import bacc

    if compute_dt == "bf16":
        in_dt = mybir.dt.bfloat16
        mm_cast = lambda ap: ap
    elif compute_dt == "f32r":
        in_dt = mybir.dt.float32
        mm_cast = lambda ap: ap.bitcast(mybir.dt.float32r)
    else:
        raise ValueError(compute_dt)

    f32 = mybir.dt.float32
    add = mybir.AluOpType.add

    K_TILES = D // P  # 4
    WAVE_NB = NB

    nc = bacc.Bacc("TRN2", target_bir_lowering=False, debug=False, num_devices=N_CORES)

    qp = nc.dram_tensor("qp", (D, ROWS + N_P), in_dt, kind="ExternalInput")
    ab = nc.dram_tensor("ab", (P, M_TILES), f32, kind="ExternalInput")
    bb = nc.dram_tensor("bb", (1, N_P), f32, kind="ExternalInput")
    out = nc.dram_tensor("out", (ROWS, N_P), f32, kind="ExternalOutput")
    N_FRONT = ROWS + WAVE_NB  # 1536
    N_REST = N_P - 2 * WAVE_NB  # 3072

    with tile.TileContext(nc) as tc:
        with (
            tc.tile_pool(name="inputs", bufs=1) as in_pool,
            tc.tile_pool(name="outs", bufs=8) as out_pool,
            tc.tile_pool(name="psum", bufs=8, space="PSUM") as psum_pool,
        ):
            qt_tiles = [None] * K_TILES
            ptb = [[None] * K_TILES for _ in range(NBLK)]

            def load_front(k):
                fr_t = in_pool.tile([P, N_FRONT], in_dt, name=f"front_{k}")
                nc.sync.dma_start(out=fr_t, in_=qp[k * P : (k + 1) * P, 0:N_FRONT])
                qt_tiles[k] = fr_t[:, 0:ROWS]
                ptb[0][k] = fr_t[:, ROWS:N_FRONT]

            def load_b1(k):
                b1_t = in_pool.tile([P, WAVE_NB], in_dt, name=f"b1_{k}")
                nc.sync.dma_start(
                    out=b1_t, in_=qp[k * P : (k + 1) * P, N_FRONT : N_FRONT + WAVE_NB]
                )
                ptb[1][k] = b1_t

            def load_rest(k):
                re_t = in_pool.tile([P, N_REST], in_dt, name=f"rest_{k}")
                nc.sync.dma_start(
                    out=re_t,
                    in_=qp[k * P : (k + 1) * P, N_FRONT + WAVE_NB : ROWS + N_P],
                )
                for b in range(2, NBLK):
                    ptb[b][k] = re_t[:, (b - 2) * WAVE_NB : (b - 1) * WAVE_NB]

            load_front(0)
            brow_t = in_pool.tile([1, N_P], f32, name="brow_t")
            nc.sync.dma_start(out=brow_t, in_=bb[0:1, :])
            bb_t = in_pool.tile([P, N_P], f32, name="bb_t")
            nc.gpsimd.partition_broadcast(bb_t, brow_t)
            for k in range(1, K_TILES):
                load_front(k)
            for k in range(K_TILES):
                load_b1(k)
            ab_t = in_pool.tile([P, M_TILES], f32, name="ab_t")
            nc.sync.dma_start(out=ab_t, in_=ab[:, :])
            for k in range(K_TILES):
                load_rest(k)

            n_out = 0

            def epilogue(psum_t, m, b):
                nonlocal n_out
                out_t = out_pool.tile([P, WAVE_NB], f32, name="out_t")
                nc.vector.scalar_tensor_tensor(
                    out=out_t,
                    in0=psum_t,
                    scalar=ab_t[:, m : m + 1],
                    in1=bb_t[:, b * WAVE_NB : (b + 1) * WAVE_NB],
                    op0=add,
                    op1=add,
                )
                out_eng = nc.scalar if n_out % 2 == 0 else nc.sync
                n_out += 1
                out_eng.dma_start(
                    out=out[m * P : (m + 1) * P, b * WAVE_NB : (b + 1) * WAVE_NB],
                    in_=out_t,
                )

            def mm(psum_t, m, b, k):
                nc.tensor.matmul(
                    psum_t,
                    mm_cast(qt_tiles[k][:, m * P : (m + 1) * P]),
                    mm_cast(ptb[b][k]),
                    start=(k == 0),
                    stop=(k == K_TILES - 1),
                )

            psums = [
                psum_pool.tile([P, WAVE_NB], f32, name="ps", tag="ps")
                for _ in range(M_TILES)
            ]
            for k in range(K_TILES):
                for m in range(M_TILES):
                    mm(psums[m], m, 0, k)
            for m in range(M_TILES):
                epilogue(psums[m], m, 0)

            for b in range(1, NBLK):
                for m in range(M_TILES):
                    psum_t = psum_pool.tile([P, WAVE_NB], f32, name="ps", tag="ps")
                    for k in range(K_TILES):
                        mm(psum_t, m, b, k)
                    epilogue(psum_t, m, b)

    nc.compile()
    return nc


def _prep_inputs_bf16(z_queries, class_prototypes, compute_dt):
    import ml_dtypes

    np_in = ml_dtypes.bfloat16 if compute_dt == "bf16" else np.float32

    z = np.ascontiguousarray(z_queries, dtype=np.float32)
    p = np.ascontiguousarray(class_prototypes, dtype=np.float32)

    a = (z.astype(np.float64) ** 2).sum(axis=1) / D
    b = (p.astype(np.float64) ** 2).sum(axis=1) / D

    pt = (p.T * np.float32(-2.0 / D)).astype(np_in)
    bbv = np.ascontiguousarray(b.astype(np.float32).reshape(1, N_P))

    in_maps = []
    for c in range(N_CORES):
        sl = slice(c * ROWS, (c + 1) * ROWS)
        qt_c = z[sl].T.astype(np_in)
        qp_c = np.ascontiguousarray(np.concatenate([qt_c, pt], axis=1))
        ab_c = np.ascontiguousarray(
            a[sl].astype(np.float32).reshape(M_TILES, P).T
        )
        in_maps.append({"qp": qp_c, "ab": ab_c, "bb": bbv})
    return in_maps


def _get_nc(compute_dt: str):
    if compute_dt not in _CACHE:
        if compute_dt == "fp8":
            _CACHE[compute_dt] = _build_nc_fp8()
        else:
            _CACHE[compute_dt] = _build_nc_bf16(compute_dt)
    return _CACHE[compute_dt]


def run(z_queries, class_prototypes, compute_dt=COMPUTE_DT, **spmd_kwargs):
    from concourse.bass_utils import run_bass_kernel_spmd

    nc = _get_nc(compute_dt)
    if compute_dt == "fp8":
        in_maps, b = _prep_inputs_fp8(z_queries, class_prototypes)
        res = run_bass_kernel_spmd(
            nc, in_maps, core_ids=list(range(N_CORES)), **spmd_kwargs
        )
        full = _finish_fp8(res, b)
    else:
        in_maps = _prep_inputs_bf16(z_queries, class_prototypes, compute_dt)
        res = run_bass_kernel_spmd(
            nc, in_maps, core_ids=list(range(N_CORES)), **spmd_kwargs
        )
        full = np.concatenate([r["out"] for r in res.results], axis=0)
    return full, res


def kernel(z_queries: np.ndarray, class_prototypes: np.ndarray) -> np.ndarray:
    full, _ = run(z_queries, class_prototypes)
    return full
